# revision 19
# baseline (speedup 1.0000x reference)
"""Trainium2 Bass kernel for nn_DenseModel_51926154609008 (weighted-rank
contrastive CE loss).

Math (reference semantics, no sort needed):
  scores = q @ p.T                       [B=2048, P=16384]
  t_i    = scores[i, 8*i]                (positive/target score)
  rank_i = #{j : scores[i, j] > t_i}     (argsort position == exceed count,
                                          ties are measure-zero for randn data)
  lse_i  = logsumexp(scores[i, :])
  loss   = mean((lse_i - t_i) * (1 + 2.6*exp(-(rank_i-1)^2 / (2*1.8^2))))

Sharding: passage-parallel (P split across 8 cores, q replicated). Each core
computes a [2048, 2048] score slab as 16 m-tiles of [128 queries x 2048
passages] (one 4-bank PSUM buffer each) and reduces every m-tile to
per-query partials:
  sumexp_c[i] = sum_j exp(s_ij - C)      (fixed shift C so partials add
                                          across cores without a max-merge)
  cnt_c[i]    = #{j in slab : s_ij > t_i}
The host combines partials and evaluates the tiny [2048] tail in fp64.

Engine plan (measured on HW, per m-tile / total):
  - PE: 12 fp8-e4m3 DoubleRow MMs ([128,2,128] @ [128,2,512] -> [128,512],
    one LDWEIGHTS per kk, the other 3 MMs set ldweights=False and reuse the
    loaded weights). 216ns/MM stream, LDW hidden -> 2.6us/tile, 41.5us.
  - ACT: ONE [128,2048] exp(s - C) + sum-accumulator spanning 4 PSUM banks
    (~2.2us/tile, ~35us). ACT is the only PSUM reader, so PSUM slot release
    = ACT completion; DVE reads ACT's bf16 output instead.
  - DVE: per-half [128,1024] count ops on je (SBUF bf16):
    #(s > t) == #(exp(s-C) > exp(t-C)) by monotonicity. exp underflow
    (s-C < -87 flushes to 0) only corrupts counts for queries whose t is
    far below the row top — their true rank is huge either way and the
    Gaussian rank weight is exactly 1 (verified numerically: loss identical).
  - Warmup during the input-DMA gap: a short fp8 DR matmul chain (PE pstate
    ramp: first real MMs otherwise run at 427-659ns instead of 216ns) and
    one junk Exp (pulls the 1.28us ACT table load off the critical path).

End-to-end fp8 quantization error on this exact input is rel 3.4e-4 on the
loss (vs the 2e-2 gate); bf16 je rounding adds nothing measurable.

The self-comparison (j == 8i) must contribute exactly 0 to rank_i. Each core
rotates its query order (data-level permutation, program stays SPMD-uniform)
so its own queries land on m-tiles OWN_M, OWN_M+1; the self columns sit at
half 0 col 8r of tile OWN_M and half 1 col 8r of tile OWN_M+1, masked in the
count via one fused scalar_tensor_tensor (shared [128,1024] mask).

t itself is computed on the host (trivial 2048x768 row-dot, exact fp32).

Other HW notes baked in from trace evidence:
  - GPSIMD/Pool cannot read PSUM (BIR verifier rejects).
  - Matmul output cannot cross a PSUM bank boundary (<=512 fp32 cols).
  - DMA issue is ~0.6us per dma_start on the issuing sequencer; chunk
    big (one dma_start spreads its descriptors over all 16 queues), order
    by first use, split across sync/scalar/gpsimd sequencers.
  - sync-hwdge lagged the final output DMA by ~2.5us; gpsimd SWDGE fires
    ~20ns after the producer -> cnt via gpsimd, se via scalar.
  - The last m-tile's stats run per-half so only half the drain is exposed
    after the final MM.
"""

import sys

import numpy as np

sys.path.insert(0, "/opt/trn_rl_repo")

import concourse.bacc as bacc  # noqa: E402
import concourse.bass as bass  # noqa: E402
import concourse.mybir as mybir  # noqa: E402
import concourse.tile as tile  # noqa: E402
from concourse.bass_utils import run_bass_kernel_spmd  # noqa: E402

# Problem shape (hardcoded per the task contract).
B = 2048
D = 768
NP = 8
P = B * NP  # 16384
NCORES = 8
PSLAB = P // NCORES  # 2048 passage columns per core
KP = D // 256  # 3 DoubleRow k-pairs (each = two 128-deep chunks)
MT = B // 128  # 16 query m-tiles of [128, PSLAB]
QSLAB = B // NCORES  # 256 queries owned per core
OWN_M = 8  # own queries sit at m-tiles 8,9
# (m, half) pairs whose count must mask the self column
MASK_HALVES = {(OWN_M, 0), (OWN_M + 1, 1)}

C_SHIFT = 128.0  # fixed exp shift: exp(s - C) never overflows for this data

ALPHA = 2.6
OPTIMAL_RANK = 1.0
SIGMA = 1.8

# se_sb cols: m=0..14 full-tile accum; 15,16 = last tile's halves
SE_COLS = MT + 1
# cnt_sb cols: 2*m + half
CNT_COLS = 2 * MT

_STATE: dict = {}


def _set_no_ldweights(mm):
    """Mark an InstMatmult as non-self-loading: it reuses the PE weights
    loaded by the immediately preceding matmul (identical lhsT AP). PE
    instructions execute in program order, so the pairing is stable."""
    mm.ins.ldweights = False


def _build_nc():
    nc = bacc.Bacc("TRN2", target_bir_lowering=False, debug=False,
                   num_devices=NCORES)

    fp8 = mybir.dt.float8e4
    f32 = mybir.dt.float32
    bf16 = mybir.dt.bfloat16
    DR = mybir.MatmulPerfMode.DoubleRow

    qT_d = nc.dram_tensor("qT", [KP, 2, 128, B], fp8, kind="ExternalInput").ap()
    pT_d = nc.dram_tensor("pT", [KP, 2, 128, PSLAB], fp8,
                          kind="ExternalInput").ap()
    tv_d = nc.dram_tensor("expt", [128, MT], f32, kind="ExternalInput").ap()
    msk_d = nc.dram_tensor("msk", [128, 1024], bf16, kind="ExternalInput").ap()
    se_d = nc.dram_tensor("se_out", [128, SE_COLS], f32,
                          kind="ExternalOutput").ap()
    cnt_d = nc.dram_tensor("cnt_out", [128, CNT_COLS], f32,
                           kind="ExternalOutput").ap()

    with tile.TileContext(nc) as tc:
        with (
            tc.tile_pool(name="weights", bufs=1) as wpool,
            tc.tile_pool(name="stats", bufs=1) as spool,
            tc.tile_pool(name="junk", bufs=3) as jpool,
            tc.tile_pool(name="psum", bufs=2,
                         space=bass.MemorySpace.PSUM) as ppool,
        ):
            qk = [wpool.tile([128, 2, B], fp8, name=f"qk{k}", tag=f"qk{k}")
                  for k in range(KP)]
            pk = [wpool.tile([128, 2, PSLAB], fp8, name=f"pk{k}", tag=f"pk{k}")
                  for k in range(KP)]
            tv = spool.tile([128, MT], f32, name="tv", tag="tv")
            msk = spool.tile([128, 1024], bf16, name="msk", tag="msk")
            negc = spool.tile([128, 1], f32, name="negc", tag="negc")
            wtile = spool.tile([128, 2, 512], fp8, name="wtile", tag="wtile")
            se_sb = spool.tile([128, SE_COLS], f32, name="se_sb", tag="se_sb")
            cnt_sb = spool.tile([128, CNT_COLS], f32, name="cnt_sb",
                                tag="cnt_sb")

            # engine-local preamble work (runs during the NEFF preamble /
            # input-DMA window): consts + warmup
            nc.vector.memset(negc[:], -C_SHIFT)
            nc.vector.memset(wtile[:], 0.0)

            def ldq(eng, kk, i, c0, c1):
                eng.dma_start(qk[kk][:, i, c0:c1], qT_d[kk, i, :, c0:c1])

            def ldp(eng, kk, i):
                eng.dma_start(pk[kk][:, i, :], pT_d[kk, i, :, :])

            # m-tile 0 consumes the whole pk (all kk, both i) within its
            # first 2.6us, so pk ships as 6 big [128,2048] chunks (one
            # dma_start spreads over all 16 queues), split across the sync
            # and scalar sequencers. qk m-tiles are consumed one per 2.6us:
            # small first chunk, big rest, on gpsimd.
            ldp(nc.sync, 0, 0)
            ldp(nc.sync, 1, 0)
            ldp(nc.sync, 2, 0)
            nc.sync.dma_start(tv[:], tv_d[:])
            ldp(nc.scalar, 0, 1)
            ldp(nc.scalar, 1, 1)
            ldp(nc.scalar, 2, 1)
            nc.scalar.dma_start(msk[:], msk_d[:])
            for kk in range(KP):
                ldq(nc.gpsimd, kk, 0, 0, 256)
                ldq(nc.gpsimd, kk, 1, 0, 256)
            for kk in range(KP):
                ldq(nc.gpsimd, kk, 0, 256, 2048)
                ldq(nc.gpsimd, kk, 1, 256, 2048)

            # PE pstate + ACT exp-table warmup on junk data while the input
            # DMAs land. 8 DR matmuls (~1.7us) + one Exp (table load 1.28us
            # + ~0.5us op). No accum_out (skips the read-accumulator tail).
            pswarm = ppool.tile([128, 2048], f32, name="pswarm", tag="ps")
            jwarm = jpool.tile([128, 2048], bf16, name="jwarm", tag="je")
            for w in range(8):
                mm = nc.tensor.matmul(
                    pswarm[:, 0:512],
                    wtile[:, :, 0:128],
                    wtile[:, :, 0:512],
                    start=(w == 0),
                    stop=(w == 7),
                    perf_mode=DR,
                )
                if w > 0:
                    _set_no_ldweights(mm)
            nc.scalar.activation(
                jwarm[:, 0:512], pswarm[:, 0:512],
                mybir.ActivationFunctionType.Exp,
                bias=negc[:], scale=1.0,
            )

            for m in range(MT):
                last = m == MT - 1
                ps = ppool.tile([128, 2048], f32, name="ps", tag="ps")
                # kk-major: LDWEIGHTS once per kk, the other three MMs
                # reuse the PE weights (ldweights=False)
                for kk in range(KP):
                    for nb in range(4):
                        mm = nc.tensor.matmul(
                            ps[:, nb * 512:(nb + 1) * 512],
                            qk[kk][:, :, m * 128:(m + 1) * 128],
                            pk[kk][:, :, nb * 512:(nb + 1) * 512],
                            start=(kk == 0),
                            stop=(kk == KP - 1),
                            perf_mode=DR,
                        )
                        if nb > 0:
                            _set_no_ldweights(mm)
                je = jpool.tile([128, 2048], bf16, name="je", tag="je")
                jc = jpool.tile([128, 2048], bf16, name="jc", tag="jc")
                # ACT: one 4-bank exp+accum per tile; the last tile splits
                # per half so only half the drain trails the final MM
                if last:
                    for h in range(2):
                        sl = slice(h * 1024, (h + 1) * 1024)
                        nc.scalar.activation(
                            je[:, sl], ps[:, sl],
                            mybir.ActivationFunctionType.Exp,
                            bias=negc[:], scale=1.0,
                            accum_out=se_sb[:, MT - 1 + h:MT + h],
                        )
                else:
                    nc.scalar.activation(
                        je[:], ps[:],
                        mybir.ActivationFunctionType.Exp,
                        bias=negc[:], scale=1.0,
                        accum_out=se_sb[:, m:m + 1],
                    )
                # DVE: per-half count on je (SBUF bf16, not PSUM)
                for h in range(2):
                    sl = slice(h * 1024, (h + 1) * 1024)
                    col = 2 * m + h
                    if (m, h) in MASK_HALVES:
                        nc.vector.scalar_tensor_tensor(
                            out=jc[:, sl], in0=je[:, sl],
                            scalar=tv[:, m:m + 1], in1=msk[:],
                            op0=mybir.AluOpType.is_gt,
                            op1=mybir.AluOpType.mult,
                            accum_out=cnt_sb[:, col:col + 1],
                        )
                    else:
                        nc.vector.tensor_scalar(
                            jc[:, sl], je[:, sl], tv[:, m:m + 1], None,
                            op0=mybir.AluOpType.is_gt,
                            op1=mybir.AluOpType.add,
                            accum_out=cnt_sb[:, col:col + 1],
                        )

            # outputs: se via the scalar hwdge (idle once the last exp is
            # done), cnt via gpsimd SWDGE (fires ~20ns after the producer)
            nc.scalar.dma_start(se_d[:], se_sb[:])
            nc.gpsimd.dma_start(cnt_d[:], cnt_sb[:])

    nc.compile()
    return nc


def _perm(c):
    """Rotation putting core c's own queries at m-tiles OWN_M, OWN_M+1."""
    return np.roll(np.arange(B), OWN_M * 128 - c * QSLAB)


def prepare(q, p):
    """Host-side shard prep. Returns (in_maps, t32, perms)."""
    import ml_dtypes
    fp8 = ml_dtypes.float8_e4m3
    q = np.ascontiguousarray(np.asarray(q, dtype=np.float32))
    p = np.ascontiguousarray(np.asarray(p, dtype=np.float32))

    # target scores t_i = q_i . p_{8i} (exact fp32; matches the reference's
    # value to ~1e-7 — only a compare threshold + host-tail term)
    t64 = np.einsum("ij,ij->i", q, p[::NP], dtype=np.float64)
    t32 = t64.astype(np.float32)
    # DVE count threshold: exp(t - C), compared against ACT's bf16 exp output
    expt = np.exp(t64 - C_SHIFT).astype(np.float32)

    qT = np.ascontiguousarray(q.T)  # [D, B] fp32
    r = np.arange(128)
    # self columns: m-tile OWN_M has query pi=OWN_M*128+r vs half-0 col 8r;
    # m-tile OWN_M+1 has pi=(OWN_M+1)*128+r vs half-1 col 8r. Same mask for
    # both, same for every core.
    msk = np.ones((128, 1024), dtype=np.float32)
    msk[r, 8 * r] = 0.0
    msk = msk.astype(ml_dtypes.bfloat16)

    in_maps = []
    perms = []
    for c in range(NCORES):
        perm = _perm(c)
        perms.append(perm)
        # [KP, 2, 128, B]: row kk*256 + i*128 + pp of qT (DoubleRow pairing)
        qTc = np.ascontiguousarray(qT[:, perm]).astype(fp8).reshape(
            KP, 2, 128, B)
        pTc = np.ascontiguousarray(
            p[c * PSLAB:(c + 1) * PSLAB].T).astype(fp8).reshape(
            KP, 2, 128, PSLAB)
        tvc = np.ascontiguousarray(expt[perm].reshape(MT, 128).T)
        in_maps.append({"qT": qTc, "pT": pTc, "expt": tvc, "msk": msk})
    return in_maps, t32, perms


def finalize(results, t32, perms):
    """Combine per-core partials into the scalar loss (fp64 host tail)."""
    se_tot = np.zeros(B, dtype=np.float64)
    cnt_tot = np.zeros(B, dtype=np.float64)
    for c in range(NCORES):
        perm = perms[c]
        # se col m (m=MT-1 split into cols MT-1, MT); cnt col 2m+h.
        # query pi = m*128 + r
        se = results[c]["se_out"].astype(np.float64)
        cnt = results[c]["cnt_out"].astype(np.float64)
        se[:, MT - 1] += se[:, MT]
        se_q = se[:, :MT].T.ravel()
        cnt_q = cnt.reshape(128, MT, 2).sum(axis=2).T.ravel()
        se_tot[perm] += se_q
        cnt_tot[perm] += cnt_q
    lse = C_SHIFT + np.log(se_tot)
    raw = lse - t32.astype(np.float64)
    w = 1.0 + ALPHA * np.exp(-((cnt_tot - OPTIMAL_RANK) ** 2)
                             / (2.0 * SIGMA ** 2))
    return np.float32(np.mean(raw * w))


def _get_nc():
    if "nc" not in _STATE:
        _STATE["nc"] = _build_nc()
    return _STATE["nc"]


def kernel(q_reps, p_reps, n_passages):
    assert int(np.asarray(n_passages)) == NP
    nc = _get_nc()
    in_maps, t32, perms = prepare(q_reps, p_reps)
    try:
        res = run_bass_kernel_spmd(nc, in_maps, core_ids=list(range(NCORES)))
    except Exception:
        # rare transient NRT_EXEC_UNIT_UNRECOVERABLE; reset the PJRT
        # client and retry once
        import time
        try:
            import jax
            jax.clear_caches()
            jax.extend.backend.clear_backends()
        except Exception:
            pass
        time.sleep(10)
        res = run_bass_kernel_spmd(nc, in_maps, core_ids=list(range(NCORES)))
    return finalize(res.results, t32, perms)


def run_profiled(q_reps, p_reps, n_passages, trace=True):
    """Same as kernel() but returns (loss, BassKernelResults) with NTFF
    profile (requires the antenv.axon_hooks shim; see _install_ntff_shim)."""
    nc = _get_nc()
    in_maps, t32, perms = prepare(q_reps, p_reps)
    res = run_bass_kernel_spmd(nc, in_maps, core_ids=list(range(NCORES)),
                               trace=trace)
    loss = finalize(res.results, t32, perms)
    return loss, res


def _install_ntff_shim():
    """Provide antenv.axon_hooks (absent in this image) so trace=True works."""
    import types
    import antenv
    if "antenv.axon_hooks" in sys.modules:
        return
    mod = types.ModuleType("antenv.axon_hooks")
    mod._hook = None
    mod.set_axon_ntff_profile_hook = lambda h: setattr(mod, "_hook", h)
    mod.get_axon_ntff_profile_hook = lambda: mod._hook
    sys.modules["antenv.axon_hooks"] = mod
    antenv.axon_hooks = mod
    try:
        from trn_agent_boot.trn_boot import _ntff_profile_via_ctypes
        hook = _ntff_profile_via_ctypes("/opt/axon/libaxon_pjrt.so")
        if hook is not None:
            mod._hook = hook
    except Exception:
        pass
